# revision 15
# baseline (speedup 1.0000x reference)
"""Neighbor aggregation (gnn message passing) Bass kernel for Trainium2.

out[b, i] = sum_{e: src[e]==i} w[e] * H[b, dst[e]]   (per batch b)

8 NeuronCores: core = 2*b + s handles batch b, src-half s (output rows
[s*25000, (s+1)*25000)).

Design (v2, matmul segment-sum — no scatter):
 - Host splits each core's ~400k edges into two streams by dst-half (so
   SWDGE gather indices fit int16 after rebasing), sorts each stream by
   src, and packs tokens into 1024-token chunks where all srcs of a chunk
   lie in a 128-row window [base, base+128).
 - H is uploaded as bf16 padded to 128 features (256B rows — the SWDGE
   gather requires elem_size % 256B == 0).
 - Per chunk: one SWDGE dma_gather (HBM -> SBUF token-major bf16),
   round-robin over the 4 SWDGE queues.  Queue q runs on Q7 cpu pair
   (2q, 2q+1), so 4 queues give ~3.4x descriptor-generation parallelism
   (the single-queue desc-gen was the baseline's 96%-busy bottleneck).
 - Per 128-token sub-chunk: one DVE tensor_scalar builds
   S2[t, s] = (iota[s] == srcrow[t]) * w[t]  (bf16, one op), then one PE
   matmul accumulates psum[s, f] += S2^T @ msgs — the segment sum with
   the edge weight folded into S2.  Unused/pad tokens carry w=0.
 - Each chunk's psum [128, 64] f32 is DMA'd to its own HBM slot; the
   host adds slots into the final output (slot src windows can overlap).
"""

import os
import sys

sys.path.insert(0, "/opt/trn_rl_repo")

import numpy as np
import ml_dtypes

import concourse.bacc as bacc
import concourse.mybir as mybir
import concourse.tile as tile
from concourse.bass_utils import run_bass_kernel_spmd

B, N, E, HS = 4, 50000, 800000, 64
NHALF = N // 2                  # 25000
C = 1024                        # tokens per chunk (HW limit per SWDGE call)
WIN = 128                       # src window (psum rows) per chunk
GRP = 8                         # chunks per grouped index/meta load
NQ = 4                          # SWDGE queues

BF16 = ml_dtypes.bfloat16

LAST_RESULT = {}


def build(nc, nchl, nchh):
    f32 = mybir.dt.float32
    bf16 = mybir.dt.bfloat16
    i16 = mybir.dt.int16

    nch = nchl + nchh
    ngrp = nch // GRP

    h_d = nc.dram_tensor("h", [2, NHALF, 128], bf16, kind="ExternalInput")
    gidx_d = nc.dram_tensor("gidx", [ngrp, 128, GRP, C // 16], i16,
                            kind="ExternalInput")
    srcw_d = nc.dram_tensor("srcw", [ngrp, 128, GRP, 2, C // 128], bf16,
                            kind="ExternalInput")
    iota_d = nc.dram_tensor("iota", [128, WIN], bf16, kind="ExternalInput")
    acc_d = nc.dram_tensor("acc", [nch, WIN, HS], f32, kind="ExternalOutput")

    KS = C // 128
    with tile.TileContext(nc) as tc:
        with tc.tile_pool(name="const", bufs=1) as cp, \
             tc.tile_pool(name="meta", bufs=4) as mp, \
             tc.tile_pool(name="work", bufs=16) as wp, \
             tc.tile_pool(name="s2p", bufs=4) as sp, \
             tc.tile_pool(name="psum", bufs=8, space="PSUM") as pp:
            iota_t = cp.tile([128, WIN], bf16, tag="iota")
            nc.sync.dma_start(iota_t[:], iota_d[:])

            gidx_g = None
            srcw_g = None
            for c in range(nch):
                st = 0 if c < nchl else 1
                g, j = c // GRP, c % GRP
                if j == 0:
                    gidx_g = mp.tile([128, GRP, C // 16], i16, tag="gidx")
                    srcw_g = mp.tile([128, GRP, 2, KS], bf16, tag="srcw")
                    nc.sync.dma_start(gidx_g[:], gidx_d[g])
                    nc.sync.dma_start(srcw_g[:], srcw_d[g])

                msgs = wp.tile([128, KS, HS], bf16, tag="msgs")
                _dma_gather_narrow(
                    nc.gpsimd,
                    out_ap=msgs[:],
                    in_ap=h_d[st][:, 0:HS],
                    idxs_ap=gidx_g[:, j],
                    num_idxs=C,
                    elem_size=HS,
                    queue_num=c % NQ,
                )

                # all 8 S2 one-hots in two whole-chunk DVE ops (per-op fixed
                # cost ~500ns dominates, so fewer/bigger ops win)
                iota_b = iota_t[:].unsqueeze(1).broadcast_to([128, KS, WIN])
                sr_b = srcw_g[:, j, 0, :].unsqueeze(2).broadcast_to(
                    [128, KS, WIN])
                w_b = srcw_g[:, j, 1, :].unsqueeze(2).broadcast_to(
                    [128, KS, WIN])
                mask = sp.tile([128, KS, WIN], bf16, tag="mask")
                nc.vector.tensor_tensor(
                    out=mask[:], in0=iota_b, in1=sr_b,
                    op=mybir.AluOpType.is_equal)
                s2 = sp.tile([128, KS, WIN], bf16, tag="s2")
                nc.vector.tensor_tensor(
                    out=s2[:], in0=mask[:], in1=w_b,
                    op=mybir.AluOpType.mult)

                pt = pp.tile([WIN, HS], f32, tag="pt")
                for k in range(KS):
                    nc.tensor.matmul(
                        pt[:],
                        s2[:, k, :],
                        msgs[:, k, :],
                        start=(k == 0),
                        stop=(k == KS - 1),
                    )
                ft = wp.tile([WIN, HS], f32, tag="ft")
                nc.scalar.copy(ft[:], pt[:])
                nc.scalar.dma_start(acc_d[c], ft[:])
    return nc


def _dma_gather_narrow(eng, out_ap, in_ap, idxs_ap, num_idxs, elem_size,
                       queue_num):
    """BassGpSimd.dma_gather (non-transpose, DRAM src) minus the
    elem_size%256B assert: the Q7 ucode only requires the row STRIDE to be a
    256B multiple (stride_bytes_256 encoding); descriptor payloads can be
    any length.  Lets us fetch 128B bf16 rows from 256B-strided storage."""
    from concourse import ap_utils
    from concourse.bass import MemorySpace

    assert idxs_ap.dtype == mybir.dt.int16
    assert in_ap.space == MemorySpace.DRAM
    assert in_ap.dtype == out_ap.dtype
    assert ap_utils.ap_is_contiguous(in_ap.ap[1:])
    assert ap_utils.ap_is_contiguous(out_ap.ap[1:])
    assert ap_utils.ap_is_contiguous(idxs_ap.ap[1:])
    assert in_ap.ap[-1][1] == out_ap.ap[-1][1] == elem_size
    assert out_ap.ap[0][1] * out_ap.ap[1][1] * 1 or True
    elem_step = in_ap.ap[0][0]
    stride_bytes = elem_step * mybir.dt.size(in_ap.dtype)
    stride_bytes_256 = stride_bytes // 256
    assert stride_bytes % 256 == 0 and stride_bytes_256 < 256

    _in_ap = eng.lower_ap_dma(in_ap, for_custom_bir_dma=True)
    _idxs_ap = eng.lower_ap(idxs_ap)
    _out_ap = eng.lower_ap(out_ap)
    return eng.add_instruction(
        mybir.InstDMAGatherAnt(
            name=eng.bass.get_next_instruction_name(),
            ins=[*_in_ap, _idxs_ap,
                 eng.lower_val_access(eng.to_reg(num_idxs))],
            outs=[_out_ap],
            transpose=False,
            num_idxs=num_idxs,
            elem_size=elem_size,
            stride_bytes_256=stride_bytes_256,
            gen_mode=0,
            single_packet=True,
            queue_num=queue_num,
            sbuf_tokens_per_rank=0,
            sbuf_free_dim_per_rank=0,
            sbuf_free_dim_pad_per_rank=0,
            sbuf_byte_offset=0,
        )
    )


_COMPILED = {}


def _get_compiled(nchl, nchh):
    key = (nchl, nchh)
    if key not in _COMPILED:
        nc = bacc.Bacc("TRN2", target_bir_lowering=False, debug=False,
                       num_swdge_queues=NQ)
        build(nc, nchl, nchh)
        nc.compile()
        _COMPILED[key] = nc
    return _COMPILED[key]


def _pack_stream(srcrow, dst_local, w):
    """Sort one stream by src and greedily pack into chunks of <= C tokens
    covering < WIN consecutive srcs.  Returns list of per-chunk
    (gidx[1024]i16, srcrow_local[1024]i16, w[1024]f32, base)."""
    order = np.argsort(srcrow, kind="stable")
    ss = srcrow[order]
    dd = dst_local[order]
    ww = w[order]
    ntok = ss.shape[0]
    if ntok == 0:
        return []
    uniq, start_idx, cnts = np.unique(ss, return_index=True, return_counts=True)
    ends = start_idx + cnts
    chunks = []
    i = 0
    nuniq = len(uniq)
    while i < nuniq:
        base = uniq[i]
        j1 = int(np.searchsorted(ends, start_idx[i] + C, side="right"))
        j2 = int(np.searchsorted(uniq, base + WIN, side="left"))
        j = min(j1, j2)
        assert j > i, f"src {base} has more than {C} edges"
        t0 = int(start_idx[i])
        t1 = int(ends[j - 1])
        chunks.append((dd[t0:t1], ss[t0:t1] - base, ww[t0:t1], int(base)))
        i = j
    return chunks


def _chunk_arrays(chunks, nch_pad):
    """Build gidx [nch,128,C//16] i16 (wrap16 layout), srcw [nch,128,2,C//128]
    bf16 (token slot p,k = token k*128+p), bases [nch]."""
    nch = len(chunks)
    assert nch <= nch_pad, (nch, nch_pad)
    # every chunk (incl. dead) carries exactly C valid idxs; pads/dead use
    # idx 0 with w=0 (gather must write every slot: 0*garbage-NaN poisons psum,
    # and a constant num_idxs_reg avoids per-chunk count registers)
    gidx = np.zeros((nch_pad, 128, C // 16), np.int16)
    srcw = np.zeros((nch_pad, 128, 2, C // 128), BF16)
    bases = np.full(nch_pad, -1, np.int64)
    for ci, (dd, sr, ww, base) in enumerate(chunks):
        n = dd.shape[0]
        # pad slots gather row 0 with w=0 (keeps every msgs slot written —
        # unwritten slots can hold NaN garbage and 0*NaN poisons the psum)
        idx = np.zeros(C, np.int64)
        idx[:n] = dd
        # wrap16: [C] -> [16, C//16] -> tile x8 -> [128, C//16]
        a = idx.reshape(C // 16, 16).T.astype(np.int16)
        gidx[ci] = np.tile(a, (8, 1))
        srz = np.zeros(C, np.float32)
        srz[:n] = sr
        wz = np.zeros(C, np.float32)
        wz[:n] = ww
        srcw[ci, :, 0, :] = srz.reshape(C // 128, 128).T.astype(BF16)
        srcw[ci, :, 1, :] = wz.reshape(C // 128, 128).T.astype(BF16)
        bases[ci] = base
    return gidx, srcw, bases


def _prep_core(src, dst, w, s):
    """Returns (chunks_low, chunks_high) for core (batch, src-half s)."""
    sel = (src >= NHALF) == bool(s)
    srcs = src[sel] - s * NHALF
    dsts = dst[sel]
    ws = w[sel]
    out = []
    for st in range(2):
        pm = (dsts >= NHALF) == bool(st)
        out.append(_pack_stream(srcs[pm], dsts[pm] - st * NHALF, ws[pm]))
    return out


def kernel(**inputs):
    H = np.asarray(inputs["H"], np.float32)
    w = np.asarray(inputs["edge_w"], np.float32)
    src = np.asarray(inputs["edge_src"], np.int64)
    dst = np.asarray(inputs["edge_dst"], np.int64)

    per_core = []
    for core in range(8):
        b, s = core // 2, core % 2
        per_core.append(_prep_core(src[b], dst[b], w[b], s))

    nchl = max(len(pc[0]) for pc in per_core)
    nchh = max(len(pc[1]) for pc in per_core)
    # pad so total is a multiple of GRP
    total = nchl + nchh
    nchh += (-total) % GRP
    nc = _get_compiled(nchl, nchh)
    nch = nchl + nchh
    ngrp = nch // GRP

    iota = np.tile(np.arange(WIN, dtype=np.float32), (128, 1)).astype(BF16)

    in_maps = []
    metas = []
    for core in range(8):
        b, s = core // 2, core % 2
        cl, chh = per_core[core]
        gl, sl, basl = _chunk_arrays(cl, nchl)
        gh, sh, bash = _chunk_arrays(chh, nchh)
        gidx = np.concatenate([gl, gh], axis=0)
        srcw = np.concatenate([sl, sh], axis=0)
        bases = np.concatenate([basl, bash], axis=0)
        # group-major layout [ngrp, 128, GRP, ...]
        gidx = np.ascontiguousarray(
            gidx.reshape(ngrp, GRP, 128, C // 16).transpose(0, 2, 1, 3))
        srcw = np.ascontiguousarray(
            srcw.reshape(ngrp, GRP, 128, 2, C // 128).transpose(0, 2, 1, 3, 4))

        hb = np.zeros((2, NHALF, 128), BF16)
        hb[0, :, :HS] = H[b, :NHALF].astype(BF16)
        hb[1, :, :HS] = H[b, NHALF:].astype(BF16)
        in_maps.append({"h": hb, "gidx": gidx, "srcw": srcw, "iota": iota})
        metas.append(bases)

    trace = bool(int(os.environ.get("GNN_TRACE", "0")))
    res = run_bass_kernel_spmd(nc, in_maps, list(range(8)), trace=trace)
    LAST_RESULT["exec_time_ns"] = res.exec_time_ns
    LAST_RESULT["res"] = res

    out = np.zeros((B, N, HS), np.float32)
    for core in range(8):
        b, s = core // 2, core % 2
        acc = np.asarray(res.results[core]["acc"], np.float32)  # [nch, WIN, HS]
        bases = metas[core]
        off = s * NHALF
        for ci in range(nch):
            base = bases[ci]
            if base < 0:
                continue
            hi = min(int(base) + WIN, NHALF)
            out[b, off + base:off + hi] += acc[ci, :hi - base]
    return out


# revision 16
# speedup vs baseline: 1.0539x; 1.0539x over previous
"""Neighbor aggregation (gnn message passing) Bass kernel for Trainium2.

out[b, i] = sum_{e: src[e]==i} w[e] * H[b, dst[e]]   (per batch b)

8 NeuronCores: core = 2*b + s handles batch b, src-half s (output rows
[s*25000, (s+1)*25000)).  ~400k edges per core.

Design (matmul segment-sum; 6.6x over the gather+scatter_add baseline):
 - Host splits each core's edges into two streams by dst-half (so SWDGE
   gather indices fit int16 after rebasing), sorts each stream by src, and
   packs tokens into 1024-token chunks whose srcs lie in a 128-row window
   [base, base+128).  Pad slots gather row 0 with w=0 (every msgs slot must
   be written: 0 * stale-NaN would poison the psum, and a constant
   num_idxs_reg avoids per-chunk count registers).
 - H is uploaded as bf16 padded to 128 features (the SWDGE gather requires
   elem_size % 256B == 0).
 - Per chunk: one SWDGE dma_gather (HBM -> SBUF token-major bf16),
   round-robin over the 4 SWDGE queues.  Queue q runs on Q7 cpu pair
   (2q, 2q+1), so 4 queues overlap descriptor generation ~2.6x (the
   single-queue desc-gen was the baseline's 96%-busy bottleneck at
   ~8ns/idx; the 4-queue steady state is ~2.7us per 1024-idx call and is
   this kernel's limiting resource).
 - Two whole-chunk DVE tensor_tensor ops build all 8 one-hot weight
   matrices S2[t, s] = (iota[s] == srcrow[t]) * w[t] (DVE per-op fixed cost
   ~500ns dominates, so 2 big ops beat 16 small ones).  8 PE matmuls
   accumulate psum[s, f] += S2^T @ msgs — the segment sum with the edge
   weight folded into S2; pad tokens carry w=0.
 - Each chunk's psum [128, 64] f32 is copied (ACT) and DMA'd to its own
   HBM slot; the host adds slots into the final output (src windows of
   neighboring chunks can overlap).
"""

import os
import sys

sys.path.insert(0, "/opt/trn_rl_repo")

import numpy as np
import ml_dtypes

import concourse.bacc as bacc
import concourse.mybir as mybir
import concourse.tile as tile
from concourse.bass_utils import run_bass_kernel_spmd

B, N, E, HS = 4, 50000, 800000, 64
NHALF = N // 2                  # 25000
C = 1024                        # tokens per chunk (HW limit per SWDGE call)
WIN = 128                       # src window (psum rows) per chunk
GRP = 8                         # chunks per grouped index/meta load
NQ = 4                          # SWDGE queues

BF16 = ml_dtypes.bfloat16

LAST_RESULT = {}


def build(nc, nchl, nchh):
    f32 = mybir.dt.float32
    bf16 = mybir.dt.bfloat16
    i16 = mybir.dt.int16

    nch = nchl + nchh
    ngrp = nch // GRP

    h_d = nc.dram_tensor("h", [2, NHALF, 128], bf16, kind="ExternalInput")
    gidx_d = nc.dram_tensor("gidx", [ngrp, 128, GRP, C // 16], i16,
                            kind="ExternalInput")
    srcw_d = nc.dram_tensor("srcw", [ngrp, 128, GRP, 2, C // 128], f32,
                            kind="ExternalInput")
    iota_d = nc.dram_tensor("iota", [128, WIN], f32, kind="ExternalInput")
    acc_d = nc.dram_tensor("acc", [nch, WIN, HS], f32, kind="ExternalOutput")

    KS = C // 128
    with tile.TileContext(nc) as tc:
        with tc.tile_pool(name="const", bufs=1) as cp, \
             tc.tile_pool(name="meta", bufs=4) as mp, \
             tc.tile_pool(name="work", bufs=16) as wp, \
             tc.tile_pool(name="s2p", bufs=4) as sp, \
             tc.tile_pool(name="psum", bufs=8, space="PSUM") as pp:
            iota_t = cp.tile([128, WIN], f32, tag="iota")
            nc.sync.dma_start(iota_t[:], iota_d[:])

            gidx_g = None
            srcw_g = None
            for c in range(nch):
                st = 0 if c < nchl else 1
                g, j = c // GRP, c % GRP
                if j == 0:
                    gidx_g = mp.tile([128, GRP, C // 16], i16, tag="gidx")
                    srcw_g = mp.tile([128, GRP, 2, KS], f32, tag="srcw")
                    nc.sync.dma_start(gidx_g[:], gidx_d[g])
                    nc.sync.dma_start(srcw_g[:], srcw_d[g])

                msgs = wp.tile([128, KS, 128], bf16, tag="msgs")
                nc.gpsimd.dma_gather(
                    out_ap=msgs[:],
                    in_ap=h_d[st],
                    idxs_ap=gidx_g[:, j],
                    num_idxs=C,
                    num_idxs_reg=C,
                    elem_size=128,
                    queue_num=c % NQ,
                )

                # all 8 S2 one-hots in two whole-chunk DVE ops (per-op fixed
                # cost ~500ns dominates, so fewer/bigger ops win)
                iota_b = iota_t[:].unsqueeze(1).broadcast_to([128, KS, WIN])
                sr_b = srcw_g[:, j, 0, :].unsqueeze(2).broadcast_to(
                    [128, KS, WIN])
                w_b = srcw_g[:, j, 1, :].unsqueeze(2).broadcast_to(
                    [128, KS, WIN])
                mask = sp.tile([128, KS, WIN], bf16, tag="mask")
                nc.vector.tensor_tensor(
                    out=mask[:], in0=iota_b, in1=sr_b,
                    op=mybir.AluOpType.is_equal)
                s2 = sp.tile([128, KS, WIN], bf16, tag="s2")
                nc.vector.tensor_tensor(
                    out=s2[:], in0=mask[:], in1=w_b,
                    op=mybir.AluOpType.mult)

                pt = pp.tile([WIN, HS], f32, tag="pt")
                for k in range(KS):
                    nc.tensor.matmul(
                        pt[:],
                        s2[:, k, :],
                        msgs[:, k, 0:HS],
                        start=(k == 0),
                        stop=(k == KS - 1),
                    )
                ft = wp.tile([WIN, HS], f32, tag="ft")
                nc.scalar.copy(ft[:], pt[:])
                nc.scalar.dma_start(acc_d[c], ft[:])
    return nc


_COMPILED = {}


def _get_compiled(nchl, nchh):
    key = (nchl, nchh)
    if key not in _COMPILED:
        nc = bacc.Bacc("TRN2", target_bir_lowering=False, debug=False,
                       num_swdge_queues=NQ)
        build(nc, nchl, nchh)
        nc.compile()
        _COMPILED[key] = nc
    return _COMPILED[key]


def _pack_stream(srcrow, dst_local, w):
    """Sort one stream by src and greedily pack into chunks of <= C tokens
    covering < WIN consecutive srcs.  Returns list of per-chunk
    (gidx[1024]i16, srcrow_local[1024]i16, w[1024]f32, base)."""
    order = np.argsort(srcrow, kind="stable")
    ss = srcrow[order]
    dd = dst_local[order]
    ww = w[order]
    ntok = ss.shape[0]
    if ntok == 0:
        return []
    uniq, start_idx, cnts = np.unique(ss, return_index=True, return_counts=True)
    ends = start_idx + cnts
    chunks = []
    i = 0
    nuniq = len(uniq)
    while i < nuniq:
        base = uniq[i]
        j1 = int(np.searchsorted(ends, start_idx[i] + C, side="right"))
        j2 = int(np.searchsorted(uniq, base + WIN, side="left"))
        j = min(j1, j2)
        assert j > i, f"src {base} has more than {C} edges"
        t0 = int(start_idx[i])
        t1 = int(ends[j - 1])
        chunks.append((dd[t0:t1], ss[t0:t1] - base, ww[t0:t1], int(base)))
        i = j
    return chunks


def _chunk_arrays(chunks, nch_pad):
    """Build gidx [nch,128,C//16] i16 (wrap16 layout), srcw [nch,128,2,C//128]
    bf16 (token slot p,k = token k*128+p), bases [nch]."""
    nch = len(chunks)
    assert nch <= nch_pad, (nch, nch_pad)
    # every chunk (incl. dead) carries exactly C valid idxs; pads/dead use
    # idx 0 with w=0 (gather must write every slot: 0*garbage-NaN poisons psum,
    # and a constant num_idxs_reg avoids per-chunk count registers)
    gidx = np.zeros((nch_pad, 128, C // 16), np.int16)
    srcw = np.zeros((nch_pad, 128, 2, C // 128), np.float32)
    bases = np.full(nch_pad, -1, np.int64)
    for ci, (dd, sr, ww, base) in enumerate(chunks):
        n = dd.shape[0]
        # pad slots gather row 0 with w=0 (keeps every msgs slot written —
        # unwritten slots can hold NaN garbage and 0*NaN poisons the psum)
        idx = np.zeros(C, np.int64)
        idx[:n] = dd
        # wrap16: [C] -> [16, C//16] -> tile x8 -> [128, C//16]
        a = idx.reshape(C // 16, 16).T.astype(np.int16)
        gidx[ci] = np.tile(a, (8, 1))
        srz = np.zeros(C, np.float32)
        srz[:n] = sr
        wz = np.zeros(C, np.float32)
        wz[:n] = ww
        srcw[ci, :, 0, :] = srz.reshape(C // 128, 128).T
        srcw[ci, :, 1, :] = wz.reshape(C // 128, 128).T
        bases[ci] = base
    return gidx, srcw, bases


def _prep_core(src, dst, w, s):
    """Returns (chunks_low, chunks_high) for core (batch, src-half s)."""
    sel = (src >= NHALF) == bool(s)
    srcs = src[sel] - s * NHALF
    dsts = dst[sel]
    ws = w[sel]
    out = []
    for st in range(2):
        pm = (dsts >= NHALF) == bool(st)
        out.append(_pack_stream(srcs[pm], dsts[pm] - st * NHALF, ws[pm]))
    return out


def kernel(**inputs):
    H = np.asarray(inputs["H"], np.float32)
    w = np.asarray(inputs["edge_w"], np.float32)
    src = np.asarray(inputs["edge_src"], np.int64)
    dst = np.asarray(inputs["edge_dst"], np.int64)

    per_core = []
    for core in range(8):
        b, s = core // 2, core % 2
        per_core.append(_prep_core(src[b], dst[b], w[b], s))

    nchl = max(len(pc[0]) for pc in per_core)
    nchh = max(len(pc[1]) for pc in per_core)
    # pad so total is a multiple of GRP
    total = nchl + nchh
    nchh += (-total) % GRP
    nc = _get_compiled(nchl, nchh)
    nch = nchl + nchh
    ngrp = nch // GRP

    iota = np.tile(np.arange(WIN, dtype=np.float32), (128, 1))

    in_maps = []
    metas = []
    for core in range(8):
        b, s = core // 2, core % 2
        cl, chh = per_core[core]
        gl, sl, basl = _chunk_arrays(cl, nchl)
        gh, sh, bash = _chunk_arrays(chh, nchh)
        gidx = np.concatenate([gl, gh], axis=0)
        srcw = np.concatenate([sl, sh], axis=0)
        bases = np.concatenate([basl, bash], axis=0)
        # group-major layout [ngrp, 128, GRP, ...]
        gidx = np.ascontiguousarray(
            gidx.reshape(ngrp, GRP, 128, C // 16).transpose(0, 2, 1, 3))
        srcw = np.ascontiguousarray(
            srcw.reshape(ngrp, GRP, 128, 2, C // 128).transpose(0, 2, 1, 3, 4))

        hb = np.zeros((2, NHALF, 128), BF16)
        hb[0, :, :HS] = H[b, :NHALF].astype(BF16)
        hb[1, :, :HS] = H[b, NHALF:].astype(BF16)
        in_maps.append({"h": hb, "gidx": gidx, "srcw": srcw, "iota": iota})
        metas.append(bases)

    trace = bool(int(os.environ.get("GNN_TRACE", "0")))
    res = run_bass_kernel_spmd(nc, in_maps, list(range(8)), trace=trace)
    LAST_RESULT["exec_time_ns"] = res.exec_time_ns
    LAST_RESULT["res"] = res

    out = np.zeros((B, N, HS), np.float32)
    for core in range(8):
        b, s = core // 2, core % 2
        acc = np.asarray(res.results[core]["acc"], np.float32)  # [nch, WIN, HS]
        bases = metas[core]
        off = s * NHALF
        for ci in range(nch):
            base = bases[ci]
            if base < 0:
                continue
            hi = min(int(base) + WIN, NHALF)
            out[b, off + base:off + hi] += acc[ci, :hi - base]
    return out


# revision 17
# speedup vs baseline: 1.0688x; 1.0141x over previous
"""Neighbor aggregation (gnn message passing) Bass kernel for Trainium2.

out[b, i] = sum_{e: src[e]==i} w[e] * H[b, dst[e]]   (per batch b)

8 NeuronCores: core = 2*b + s handles batch b, src-half s (output rows
[s*25000, (s+1)*25000)).  ~400k edges per core.

Design (matmul segment-sum; 6.6x over the gather+scatter_add baseline):
 - Host splits each core's edges into two streams by dst-half (so SWDGE
   gather indices fit int16 after rebasing), sorts each stream by src, and
   packs tokens into 1024-token chunks whose srcs lie in a 128-row window
   [base, base+128).  Pad slots gather row 0 with w=0 (every msgs slot must
   be written: 0 * stale-NaN would poison the psum, and a constant
   num_idxs_reg avoids per-chunk count registers).
 - H is uploaded as bf16 padded to 128 features (the SWDGE gather requires
   elem_size % 256B == 0).
 - Per chunk: one SWDGE dma_gather (HBM -> SBUF token-major bf16),
   round-robin over the 4 SWDGE queues.  Queue q runs on Q7 cpu pair
   (2q, 2q+1), so 4 queues overlap descriptor generation ~2.6x (the
   single-queue desc-gen was the baseline's 96%-busy bottleneck at
   ~8ns/idx; the 4-queue steady state is ~2.7us per 1024-idx call and is
   this kernel's limiting resource).
 - Two whole-chunk DVE tensor_tensor ops build all 8 one-hot weight
   matrices S2[t, s] = (iota[s] == srcrow[t]) * w[t] (DVE per-op fixed cost
   ~500ns dominates, so 2 big ops beat 16 small ones).  8 PE matmuls
   accumulate psum[s, f] += S2^T @ msgs — the segment sum with the edge
   weight folded into S2; pad tokens carry w=0.
 - Each chunk's psum [128, 64] f32 is copied (ACT) and DMA'd to its own
   HBM slot; the host adds slots into the final output (src windows of
   neighboring chunks can overlap).
"""

import os
import sys

sys.path.insert(0, "/opt/trn_rl_repo")

import numpy as np
import ml_dtypes

import concourse.bacc as bacc
import concourse.mybir as mybir
import concourse.tile as tile
from concourse.bass_utils import run_bass_kernel_spmd

B, N, E, HS = 4, 50000, 800000, 64
NHALF = N // 2                  # 25000
C = 1024                        # tokens per chunk (HW limit per SWDGE call)
WIN = 128                       # src window (psum rows) per chunk
GRP = 8                         # chunks per grouped index/meta load
NQ = 4                          # SWDGE queues

BF16 = ml_dtypes.bfloat16

LAST_RESULT = {}


def build(nc, nchl, nchh):
    f32 = mybir.dt.float32
    bf16 = mybir.dt.bfloat16
    i16 = mybir.dt.int16

    nch = nchl + nchh
    ngrp = nch // GRP

    h_d = nc.dram_tensor("h", [2, NHALF, 128], bf16, kind="ExternalInput")
    gidx_d = nc.dram_tensor("gidx", [ngrp, 128, GRP, C // 16], i16,
                            kind="ExternalInput")
    srcw_d = nc.dram_tensor("srcw", [ngrp, 128, GRP, 2, C // 128], f32,
                            kind="ExternalInput")
    iota_d = nc.dram_tensor("iota", [128, WIN], f32, kind="ExternalInput")
    acc_d = nc.dram_tensor("acc", [nch, WIN, HS], f32, kind="ExternalOutput")

    KS = C // 128
    with tile.TileContext(nc) as tc:
        with tc.tile_pool(name="const", bufs=1) as cp, \
             tc.tile_pool(name="meta", bufs=6) as mp, \
             tc.tile_pool(name="work", bufs=28) as wp, \
             tc.tile_pool(name="s2p", bufs=8) as sp, \
             tc.tile_pool(name="psum", bufs=8, space="PSUM") as pp:
            iota_t = cp.tile([128, WIN], f32, tag="iota")
            nc.sync.dma_start(iota_t[:], iota_d[:])

            gidx_g = None
            srcw_g = None
            for c in range(nch):
                st = 0 if c < nchl else 1
                g, j = c // GRP, c % GRP
                if j == 0:
                    gidx_g = mp.tile([128, GRP, C // 16], i16, tag="gidx")
                    srcw_g = mp.tile([128, GRP, 2, KS], f32, tag="srcw")
                    nc.sync.dma_start(gidx_g[:], gidx_d[g])
                    nc.sync.dma_start(srcw_g[:], srcw_d[g])

                msgs = wp.tile([128, KS, 128], bf16, tag="msgs")
                nc.gpsimd.dma_gather(
                    out_ap=msgs[:],
                    in_ap=h_d[st],
                    idxs_ap=gidx_g[:, j],
                    num_idxs=C,
                    num_idxs_reg=C,
                    elem_size=128,
                    queue_num=c % NQ,
                )

                # all 8 S2 one-hots in two whole-chunk DVE ops (per-op fixed
                # cost ~500ns dominates, so fewer/bigger ops win)
                iota_b = iota_t[:].unsqueeze(1).broadcast_to([128, KS, WIN])
                sr_b = srcw_g[:, j, 0, :].unsqueeze(2).broadcast_to(
                    [128, KS, WIN])
                w_b = srcw_g[:, j, 1, :].unsqueeze(2).broadcast_to(
                    [128, KS, WIN])
                mask = sp.tile([128, KS, WIN], bf16, tag="mask")
                nc.vector.tensor_tensor(
                    out=mask[:], in0=iota_b, in1=sr_b,
                    op=mybir.AluOpType.is_equal)
                s2 = sp.tile([128, KS, WIN], bf16, tag="s2")
                nc.vector.tensor_tensor(
                    out=s2[:], in0=mask[:], in1=w_b,
                    op=mybir.AluOpType.mult)

                pt = pp.tile([WIN, HS], f32, tag="pt")
                for k in range(KS):
                    nc.tensor.matmul(
                        pt[:],
                        s2[:, k, :],
                        msgs[:, k, 0:HS],
                        start=(k == 0),
                        stop=(k == KS - 1),
                    )
                ft = wp.tile([WIN, HS], f32, tag="ft")
                nc.scalar.copy(ft[:], pt[:])
                nc.scalar.dma_start(acc_d[c], ft[:])
    return nc


_COMPILED = {}


def _get_compiled(nchl, nchh):
    key = (nchl, nchh)
    if key not in _COMPILED:
        nc = bacc.Bacc("TRN2", target_bir_lowering=False, debug=False,
                       num_swdge_queues=NQ)
        build(nc, nchl, nchh)
        nc.compile()
        _COMPILED[key] = nc
    return _COMPILED[key]


def _pack_stream(srcrow, dst_local, w):
    """Sort one stream by src and greedily pack into chunks of <= C tokens
    covering < WIN consecutive srcs.  Returns list of per-chunk
    (gidx[1024]i16, srcrow_local[1024]i16, w[1024]f32, base)."""
    order = np.argsort(srcrow, kind="stable")
    ss = srcrow[order]
    dd = dst_local[order]
    ww = w[order]
    ntok = ss.shape[0]
    if ntok == 0:
        return []
    uniq, start_idx, cnts = np.unique(ss, return_index=True, return_counts=True)
    ends = start_idx + cnts
    chunks = []
    i = 0
    nuniq = len(uniq)
    while i < nuniq:
        base = uniq[i]
        j1 = int(np.searchsorted(ends, start_idx[i] + C, side="right"))
        j2 = int(np.searchsorted(uniq, base + WIN, side="left"))
        j = min(j1, j2)
        assert j > i, f"src {base} has more than {C} edges"
        t0 = int(start_idx[i])
        t1 = int(ends[j - 1])
        chunks.append((dd[t0:t1], ss[t0:t1] - base, ww[t0:t1], int(base)))
        i = j
    return chunks


def _chunk_arrays(chunks, nch_pad):
    """Build gidx [nch,128,C//16] i16 (wrap16 layout), srcw [nch,128,2,C//128]
    bf16 (token slot p,k = token k*128+p), bases [nch]."""
    nch = len(chunks)
    assert nch <= nch_pad, (nch, nch_pad)
    # every chunk (incl. dead) carries exactly C valid idxs; pads/dead use
    # idx 0 with w=0 (gather must write every slot: 0*garbage-NaN poisons psum,
    # and a constant num_idxs_reg avoids per-chunk count registers)
    gidx = np.zeros((nch_pad, 128, C // 16), np.int16)
    srcw = np.zeros((nch_pad, 128, 2, C // 128), np.float32)
    bases = np.full(nch_pad, -1, np.int64)
    for ci, (dd, sr, ww, base) in enumerate(chunks):
        n = dd.shape[0]
        # pad slots gather row 0 with w=0 (keeps every msgs slot written —
        # unwritten slots can hold NaN garbage and 0*NaN poisons the psum)
        idx = np.zeros(C, np.int64)
        idx[:n] = dd
        # wrap16: [C] -> [16, C//16] -> tile x8 -> [128, C//16]
        a = idx.reshape(C // 16, 16).T.astype(np.int16)
        gidx[ci] = np.tile(a, (8, 1))
        srz = np.zeros(C, np.float32)
        srz[:n] = sr
        wz = np.zeros(C, np.float32)
        wz[:n] = ww
        srcw[ci, :, 0, :] = srz.reshape(C // 128, 128).T
        srcw[ci, :, 1, :] = wz.reshape(C // 128, 128).T
        bases[ci] = base
    return gidx, srcw, bases


def _prep_core(src, dst, w, s):
    """Returns (chunks_low, chunks_high) for core (batch, src-half s)."""
    sel = (src >= NHALF) == bool(s)
    srcs = src[sel] - s * NHALF
    dsts = dst[sel]
    ws = w[sel]
    out = []
    for st in range(2):
        pm = (dsts >= NHALF) == bool(st)
        out.append(_pack_stream(srcs[pm], dsts[pm] - st * NHALF, ws[pm]))
    return out


def kernel(**inputs):
    H = np.asarray(inputs["H"], np.float32)
    w = np.asarray(inputs["edge_w"], np.float32)
    src = np.asarray(inputs["edge_src"], np.int64)
    dst = np.asarray(inputs["edge_dst"], np.int64)

    per_core = []
    for core in range(8):
        b, s = core // 2, core % 2
        per_core.append(_prep_core(src[b], dst[b], w[b], s))

    nchl = max(len(pc[0]) for pc in per_core)
    nchh = max(len(pc[1]) for pc in per_core)
    # pad so total is a multiple of GRP
    total = nchl + nchh
    nchh += (-total) % GRP
    nc = _get_compiled(nchl, nchh)
    nch = nchl + nchh
    ngrp = nch // GRP

    iota = np.tile(np.arange(WIN, dtype=np.float32), (128, 1))

    in_maps = []
    metas = []
    for core in range(8):
        b, s = core // 2, core % 2
        cl, chh = per_core[core]
        gl, sl, basl = _chunk_arrays(cl, nchl)
        gh, sh, bash = _chunk_arrays(chh, nchh)
        gidx = np.concatenate([gl, gh], axis=0)
        srcw = np.concatenate([sl, sh], axis=0)
        bases = np.concatenate([basl, bash], axis=0)
        # group-major layout [ngrp, 128, GRP, ...]
        gidx = np.ascontiguousarray(
            gidx.reshape(ngrp, GRP, 128, C // 16).transpose(0, 2, 1, 3))
        srcw = np.ascontiguousarray(
            srcw.reshape(ngrp, GRP, 128, 2, C // 128).transpose(0, 2, 1, 3, 4))

        hb = np.zeros((2, NHALF, 128), BF16)
        hb[0, :, :HS] = H[b, :NHALF].astype(BF16)
        hb[1, :, :HS] = H[b, NHALF:].astype(BF16)
        in_maps.append({"h": hb, "gidx": gidx, "srcw": srcw, "iota": iota})
        metas.append(bases)

    trace = bool(int(os.environ.get("GNN_TRACE", "0")))
    res = run_bass_kernel_spmd(nc, in_maps, list(range(8)), trace=trace)
    LAST_RESULT["exec_time_ns"] = res.exec_time_ns
    LAST_RESULT["res"] = res

    out = np.zeros((B, N, HS), np.float32)
    for core in range(8):
        b, s = core // 2, core % 2
        acc = np.asarray(res.results[core]["acc"], np.float32)  # [nch, WIN, HS]
        bases = metas[core]
        off = s * NHALF
        for ci in range(nch):
            base = bases[ci]
            if base < 0:
                continue
            hi = min(int(base) + WIN, NHALF)
            out[b, off + base:off + hi] += acc[ci, :hi - base]
    return out
